# revision 1
# baseline (speedup 1.0000x reference)
"""Trainium2 Bass kernel for nn_CRF: dense layer + Viterbi decode.

Problem: inputs [64, 512, 1024] @ kernel [1024, 128] + bias -> logits
[64, 512, 128]; Viterbi max-plus forward scan over T=512 with transition
matrix chain_kernel [128, 128]; backtrace -> tags [64, 512] (float32).

Sharding: data-parallel over batch across 8 NeuronCores (8 rows each).

Per-core design (b = 8 local batch rows, U = 128 tags, T = 512):
  Phase 1  PE matmul (8 K-chunks, fp32 PSUM accumulation) produces
           pot_T [128(u), T*8] in SBUF with free index t*8+b.
  Phase 2  forward scan, partitions = j (next tag):
           8x tensor_tensor_reduce (add+max fused) per step against a
           state tensor replicated across partitions via PE matmuls
           (transpose -> ACT copy -> 8 selector matmuls), pot-add writes
           states_j [128(j), T*8].
  Phase 3  backtrace, partitions = 16 replicas per batch row
           (p = 16*b + r), so the per-16-partition-group shared-index
           semantics of gpsimd indirect_copy give a per-row gather of
           chain columns; fused TTR add+max then vector.max_index
           (first-index tie-break, matching jnp.argmax).
  Output   tags_sc [128, T*8] uint16 DMA'd out; host extracts
           [::16, ::8] and casts to float32.

All scan arithmetic is elementwise fp32 identical to the reference ops,
so decisions match the reference bit-for-bit given the same logits; the
only divergence source is fp32 matmul summation order (~1e-6), measured
to flip ~0 of 32768 tags.
"""

import os
import sys

for _p in ("/opt/trn_rl_repo",):
    if _p not in sys.path:
        sys.path.append(_p)

import numpy as np

import concourse.bacc as bacc
import concourse.mybir as mybir
import concourse.tile as tile
from concourse import bass_utils

B, T, D, U = 64, 512, 1024, 128
T = int(os.environ.get("CRF_T", T))  # dev-only override for sim tests
NCORES = 8
BL = B // NCORES          # local batch rows per core
ROWS = BL * T             # 4096 rows per core
FLT_MIN = -3.4028234663852886e38

_CACHE = {}


def _build():
    f32 = mybir.dt.float32
    u16 = mybir.dt.uint16
    ADD = mybir.AluOpType.add
    MAX = mybir.AluOpType.max

    nc = bacc.Bacc("TRN2", target_bir_lowering=False, debug=False,
                   num_devices=NCORES)

    i_xt = nc.dram_tensor("xt", [D, ROWS], f32, kind="ExternalInput").ap()
    i_wk = nc.dram_tensor("wk", [D, U], f32, kind="ExternalInput").ap()
    i_bias = nc.dram_tensor("bias", [U, 1], f32, kind="ExternalInput").ap()
    i_ct = nc.dram_tensor("ct", [U, U], f32, kind="ExternalInput").ap()
    i_cflat = nc.dram_tensor("cflat", [128, U * U], f32,
                             kind="ExternalInput").ap()
    i_sels = nc.dram_tensor("sels", [BL, BL * U], f32,
                            kind="ExternalInput").ap()
    i_sel16 = nc.dram_tensor("sel16", [BL, 128], f32,
                             kind="ExternalInput").ap()
    i_ident = nc.dram_tensor("ident", [128, 128], f32,
                             kind="ExternalInput").ap()
    i_poff = nc.dram_tensor("poff", [128, 1], u16,
                            kind="ExternalInput").ap()
    o_tags = nc.dram_tensor("tags", [128, T * BL], u16,
                            kind="ExternalOutput").ap()

    with tile.TileContext(nc) as tc:
        with tc.tile_pool(name="const", bufs=1) as cpool, \
             tc.tile_pool(name="big", bufs=1) as bpool, \
             tc.tile_pool(name="work", bufs=2) as wpool, \
             tc.tile_pool(name="sc", bufs=1) as scpool:

            ct_t = cpool.tile([U, U], f32)
            nc.sync.dma_start(out=ct_t[:], in_=i_ct[:])
            wk_t = cpool.tile([128, 8 * U], f32)
            for c in range(8):
                nc.sync.dma_start(out=wk_t[:, c * U:(c + 1) * U],
                                  in_=i_wk[c * 128:(c + 1) * 128, :])
            bias_t = cpool.tile([U, 1], f32)
            nc.sync.dma_start(out=bias_t[:], in_=i_bias[:])
            sels_t = cpool.tile([BL, BL * U], f32)
            nc.sync.dma_start(out=sels_t[:], in_=i_sels[:])
            sel16_t = cpool.tile([BL, 128], f32)
            nc.sync.dma_start(out=sel16_t[:], in_=i_sel16[:])
            ident_t = cpool.tile([128, 128], f32)
            nc.sync.dma_start(out=ident_t[:], in_=i_ident[:])
            cflat_t = cpool.tile([128, U * U], f32)
            nc.sync.dma_start(out=cflat_t[:], in_=i_cflat[:])
            poff_t = cpool.tile([128, 1], u16)
            nc.sync.dma_start(out=poff_t[:], in_=i_poff[:])

            pot = bpool.tile([U, T * BL], f32)       # free idx = t*8+b
            states = bpool.tile([U, T * BL], f32)    # free idx = t*8+b
            tags_sc = bpool.tile([128, T * BL], u16)

            # ---------------- Phase 1: logits -> pot ----------------
            with tc.tile_pool(name="xt", bufs=2) as xtpool, \
                 tc.tile_pool(name="ph1", bufs=1, space="PSUM") as ph1psum:
                ps_n = [ph1psum.tile([U, T], f32, tag=f"mm{n}", name=f"mm{n}")
                        for n in range(BL)]
                for c in range(8):
                    xt_c = xtpool.tile([128, ROWS], f32, tag="xt")
                    nc.sync.dma_start(out=xt_c[:],
                                      in_=i_xt[c * 128:(c + 1) * 128, :])
                    for n in range(BL):
                        nc.tensor.matmul(ps_n[n][:],
                                         wk_t[:, c * U:(c + 1) * U],
                                         xt_c[:, n * T:(n + 1) * T],
                                         start=(c == 0), stop=(c == 7))
                pot3 = pot[:].rearrange("p (t b) -> p t b", b=BL)
                for n in range(BL):
                    # rows of chunk n are (b=n, t): bias add on copy-out
                    nc.vector.tensor_scalar_add(out=pot3[:, :, n],
                                                in0=ps_n[n][:],
                                                scalar1=bias_t[:, 0:1])

            # -------------- Phase 2: forward max-plus scan ----------
            ph2 = tc.tile_pool(name="ph2", bufs=2, space="PSUM")
            psum = ph2.__enter__()

            def replicate(t):
                """states[:, t*8:+8] -> Ysb [8,128] and s_rep [128, 8*U]."""
                y_ps = psum.tile([BL, 128], f32, tag="y")
                nc.tensor.transpose(y_ps[:], states[:, t * BL:(t + 1) * BL],
                                    ident_t[:])
                ysb = wpool.tile([BL, 128], f32, tag="ysb")
                nc.scalar.copy(out=ysb[:], in_=y_ps[:])
                srep = psum.tile([128, BL * U], f32, tag="srep")
                for b in range(BL):
                    nc.tensor.matmul(srep[:, b * U:(b + 1) * U],
                                     sels_t[:, b * U:(b + 1) * U],
                                     ysb[:], start=True, stop=True)
                return srep

            nc.vector.tensor_copy(out=states[:, 0:BL], in_=pot[:, 0:BL])
            srep = replicate(0)
            ct_b = ct_t[:].rearrange("p (a i) -> p a i", a=1) \
                          .broadcast_to((U, BL, U))
            for t in range(1, T):
                scores = scpool.tile([U, BL * U], f32, tag="scores", bufs=2,
                                     name="scores")
                nc.vector.tensor_add(
                    out=scores[:].rearrange("p (b i) -> p b i", i=U),
                    in0=ct_b,
                    in1=srep[:].rearrange("p (b i) -> p b i", i=U))
                maxv = wpool.tile([U, BL], f32, tag="maxv")
                nc.vector.reduce_max(
                    out=maxv[:],
                    in_=scores[:].rearrange("p (b i) -> p b i", i=U),
                    axis=mybir.AxisListType.X)
                nc.vector.tensor_add(out=states[:, t * BL:(t + 1) * BL],
                                     in0=maxv[:],
                                     in1=pot[:, t * BL:(t + 1) * BL])
                if t < T - 1:
                    srep = replicate(t)

            ph2.__exit__(None, None, None)

            # -------------- Phase 3: backtrace ----------------------
            ph3 = tc.tile_pool(name="ph3", bufs=2, space="PSUM")
            psum = ph3.__enter__()

            def state_rep16(t):
                """states[:, t*8:+8] -> [128, 128] f32, row p = s_t[p//16]."""
                y_ps = psum.tile([BL, 128], f32, tag="y2")
                nc.tensor.transpose(y_ps[:], states[:, t * BL:(t + 1) * BL],
                                    ident_t[:])
                ysb = wpool.tile([BL, 128], f32, tag="ysb2")
                nc.scalar.copy(out=ysb[:], in_=y_ps[:])
                stf = psum.tile([128, 128], f32, tag="stf")
                nc.tensor.matmul(stf[:], sel16_t[:], ysb[:],
                                 start=True, stop=True)
                sts = wpool.tile([128, 128], f32, tag="sts")
                nc.scalar.copy(out=sts[:], in_=stf[:])
                return sts

            s_last = state_rep16(T - 1)
            vmax8 = wpool.tile([128, 8], f32, tag="vmax8")
            nc.vector.max(vmax8[:], s_last[:])
            nc.vector.max_index(tags_sc[:, (T - 1) * BL:T * BL],
                                vmax8[:], s_last[:])

            cflat3 = cflat_t[:].rearrange("p (j i) -> p j i", i=32)
            for t in range(T - 1, 0, -1):
                sts = state_rep16(t - 1)
                idxs = wpool.tile([128, 1], u16, tag="idxs")
                nc.vector.scalar_tensor_tensor(
                    out=idxs[:], in0=tags_sc[:, t * BL:t * BL + 1],
                    scalar=U, in1=poff_t[:],
                    op0=mybir.AluOpType.mult, op1=ADD)
                colc = wpool.tile([128, U], f32, tag="colc")
                nc.gpsimd.indirect_copy(
                    out=colc[:].rearrange("p (a i) -> p a i", i=32),
                    data=cflat3, idxs=idxs[:],
                    i_know_ap_gather_is_preferred=True)
                v = wpool.tile([128, U], f32, tag="v")
                nc.vector.tensor_add(out=v[:], in0=colc[:], in1=sts[:])
                vm8 = wpool.tile([128, 8], f32, tag="vm8")
                nc.vector.max(vm8[:], v[:])
                nc.vector.max_index(tags_sc[:, (t - 1) * BL:t * BL],
                                    vm8[:], v[:])

            ph3.__exit__(None, None, None)

            nc.sync.dma_start(out=o_tags[:], in_=tags_sc[:])

    nc.compile()
    return nc


def _prep_inputs(inputs, kernel, bias, chain_kernel):
    x = np.ascontiguousarray(inputs, dtype=np.float32)
    wk = np.ascontiguousarray(kernel, dtype=np.float32)
    bi = np.ascontiguousarray(bias, dtype=np.float32).reshape(U, 1)
    ch = np.ascontiguousarray(chain_kernel, dtype=np.float32)

    ct = np.ascontiguousarray(ch.T)                      # ct[j, i] = C[i, j]
    cflat = np.broadcast_to(ct.reshape(1, U * U), (128, U * U))
    cflat = np.ascontiguousarray(cflat)
    sels = np.zeros((BL, BL * U), np.float32)
    for b in range(BL):
        sels[b, b * U:(b + 1) * U] = 1.0
    sel16 = np.zeros((BL, 128), np.float32)
    for p in range(128):
        sel16[p // 16, p] = 1.0
    ident = np.eye(128, dtype=np.float32)
    poff = np.zeros((128, 1), np.uint16)
    for p in range(128):
        poff[p, 0] = 32 * (p % 16) if (p % 16) < 4 else 0

    in_maps = []
    for c in range(NCORES):
        shard = x[c * BL:(c + 1) * BL]                   # [8, 512, 1024]
        xt = np.ascontiguousarray(shard.reshape(ROWS, D).T)
        in_maps.append({
            "xt": xt, "wk": wk, "bias": bi, "ct": ct, "cflat": cflat,
            "sels": sels, "sel16": sel16, "ident": ident, "poff": poff,
        })
    return in_maps


def kernel(inputs, kernel, bias, chain_kernel):
    if "nc" not in _CACHE:
        _CACHE["nc"] = _build()
    nc = _CACHE["nc"]
    in_maps = _prep_inputs(inputs, kernel, bias, chain_kernel)
    res = bass_utils.run_bass_kernel_spmd(nc, in_maps,
                                          core_ids=list(range(NCORES)))
    out = np.empty((B, T), np.float32)
    for c in range(NCORES):
        raw = res.results[c]["tags"]                     # [128, T*8] u16
        out[c * BL:(c + 1) * BL] = raw[::16, ::BL].astype(np.float32)
    return out


if __name__ == "__main__":
    rng = np.random.default_rng(0)
    ins = {
        "inputs": rng.standard_normal((B, T, D)).astype(np.float32),
        "kernel": (rng.standard_normal((D, U)) / np.sqrt(D)).astype(np.float32),
        "bias": np.zeros((U,), np.float32),
        "chain_kernel": (rng.standard_normal((U, U)) * 0.1).astype(np.float32),
    }
    out = kernel(**ins)
    print(out.shape, out.dtype, out[:2, :8])

